# revision 13
# baseline (speedup 1.0000x reference)
"""Trainium2 Bass kernel for nn_Decoder (fc + 3-layer GRU + mask).

Strategy: data-parallel over batch B=32 across 8 cores (4 samples/core).
Gate-major layout: gate/hidden dims on partitions, (hc, b) on free.

v2: the per-chunk input projections (gx), gate biases, and the layer-0
fc term are all accumulated INTO PSUM by the tensor engine, so the
per-step gate chain reads pre-activations straight from PSUM:

  PSUM pg [128, 16, TC, BS] per chunk (2 banks):
    bank0 j=0..7  : gx_rz + (b_ih+b_hh)_rz + gh_rz   (step MMs accumulate)
    bank1 j=8..11 : gx_n + b_ih_n  (read-only per step)
    bank1 j=12..15: b_hh_n + gh_n                     (step MMs accumulate)

Per step: PE 32 rz-MMs -> sigma (ACT, overlaps the 16 n-MMs) ->
rn = C*r -> aN = rn + B -> tanh -> (1-z), z*h_prev under tanh's shadow
-> h = n*(1-z) + z*h_prev written directly to the fp16 step buffer.
"""

import os
import sys
from contextlib import ExitStack

for _p in ("/opt/trn_rl_repo",):
    if _p not in sys.path:
        sys.path.insert(0, _p)

import numpy as np
import ml_dtypes

import concourse.bass as bass
import concourse.bacc as bacc
import concourse.mybir as mybir
import concourse.tile as tile
from concourse import bass_utils

BF = np.float16
F32 = np.float32
dt = mybir.dt

NCORES = 8
B, T = 32, 512
BS = B // NCORES          # 4 samples per core
TC = 16                   # time-steps per chunk
NCHUNK = T // TC          # 32
HID = 1024                # layer-0 input dim
H = 512                   # GRU hidden
G3 = 3 * H                # 1536
MCH = G3 // 128           # 12 gate-dim chunks: r0..3 z0..3 n0..3
KC = H // 128             # 4 hidden chunks
KC0 = HID // 128          # 8 input chunks for layer 0
FREE = KC * BS            # 16
REPEAT = 0
ABLATE = ""               # "nochain", "nogates"
RDEV = 0                  # device-side repeats of a layer-1 pass
RFULL = 0                 # device-side repeats of the full 3-pass body

Sigmoid = mybir.ActivationFunctionType.Sigmoid
Tanh = mybir.ActivationFunctionType.Tanh
Relu = mybir.ActivationFunctionType.Relu
Alu = mybir.AluOpType


def _declare_io(nc):
    d = {}
    inp = lambda n, s, t: nc.dram_tensor(n, s, t, kind="ExternalInput").ap()
    d["chordT"] = inp("chordT", [KC0, 128, T, BS], dt.float16)
    d["zT"] = inp("zT", [2, 128, BS], dt.float16)
    d["fcwT"] = inp("fcwT", [2, 128, HID], dt.float16)
    d["fcb"] = inp("fcb", [KC0, 128, 1], dt.float32)
    d["wihT0"] = inp("wihT0", [KC0, 128, G3], dt.float16)
    d["wihT1"] = inp("wihT1", [KC, 128, G3], dt.float16)
    d["wihT2"] = inp("wihT2", [KC, 128, G3], dt.float16)
    d["whhT0"] = inp("whhT0", [KC, 128, G3], dt.float16)
    d["whhT1"] = inp("whhT1", [KC, 128, G3], dt.float16)
    d["whhT2"] = inp("whhT2", [KC, 128, G3], dt.float16)
    d["biasT"] = inp("biasT", [4, 3 * 24 * 128], dt.float16)
    d["onehotT"] = inp("onehotT", [4, TC * BS], dt.float16)
    d["iota"] = inp("iota", [128, T], dt.float32)
    d["seqrep"] = inp("seqrep", [128, BS], dt.float32)
    d["outT"] = nc.dram_tensor("outT", [128, T, KC, BS], dt.float32,
                               kind="ExternalOutput").ap()
    return d


def _build_program(debug=False):
    nc = bacc.Bacc("TRN2", target_bir_lowering=False, debug=debug,
                   num_devices=NCORES)
    io = _declare_io(nc)
    with tile.TileContext(nc) as tc:
        _emit(tc, io)
    nc.compile()
    return nc


def _emit(tc, io):
    nc = tc.nc
    ctx = ExitStack()
    const = ctx.enter_context(tc.tile_pool(name="const", bufs=1))
    stream = ctx.enter_context(tc.tile_pool(name="stream", bufs=3))
    tmp = ctx.enter_context(tc.tile_pool(name="tmp", bufs=3))
    outp = ctx.enter_context(tc.tile_pool(name="outp", bufs=2))
    pgp = ctx.enter_context(tc.tile_pool(name="pg", bufs=2, space="PSUM"))

    # ---- persistent SBUF tensors -------------------------------------
    wih = [const.tile([128, KC0, G3], dt.float16, tag="wih0", name="wih0"),
           const.tile([128, KC, G3], dt.float16, tag="wih1", name="wih1"),
           const.tile([128, KC, G3], dt.float16, tag="wih2", name="wih2")]
    whh = [const.tile([128, KC, G3], dt.float16, tag=f"whh{l}",
                      name=f"whh{l}") for l in range(3)]
    hseq = [const.tile([128, T, KC, BS], dt.float16, tag=f"hseq{i}",
                       name=f"hseq{i}") for i in range(2)]
    mask = const.tile([128, T, BS], dt.float16, tag="mask")
    biasb = const.tile([4, 3, 24, 128], dt.float16, tag="biasb")
    onehot = const.tile([4, TC * BS], dt.float16, tag="onehot")
    fc_hT = const.tile([128, KC0, BS], dt.float16, tag="fchT")
    gfcT = const.tile([4, MCH, 128], dt.float16, tag="gfcT")
    hbf = const.tile([128, TC, KC, BS], dt.float16, tag="hbf")

    # ---- load weights / constants ------------------------------------
    for kc in range(KC0):
        nc.sync.dma_start(wih[0][:, kc, :], io["wihT0"][kc])
    for l in (1, 2):
        for kc in range(KC):
            nc.sync.dma_start(wih[l][:, kc, :], io[f"wihT{l}"][kc])
    for l in range(3):
        for kc in range(KC):
            nc.sync.dma_start(whh[l][:, kc, :], io[f"whhT{l}"][kc])
    nc.sync.dma_start(
        biasb[:].rearrange("p l j g -> p (l j g)"), io["biasT"])
    nc.sync.dma_start(onehot[:], io["onehotT"])

    # ---- prologue: mask, fc, gfcT ------------------------------------
    with ExitStack() as pctx:
        psb = pctx.enter_context(tc.tile_pool(name="psb", bufs=2))
        pps = pctx.enter_context(tc.tile_pool(name="pps", bufs=1,
                                              space="PSUM"))

        iota_sb = psb.tile([128, T], dt.float32, tag="iota")
        seq_sb = psb.tile([128, BS], dt.float32, tag="seq")
        nc.sync.dma_start(iota_sb[:], io["iota"])
        nc.sync.dma_start(seq_sb[:], io["seqrep"])
        for b in range(BS):
            nc.vector.tensor_scalar(mask[:, :, b], iota_sb[:],
                                    seq_sb[:, b:b + 1], None, op0=Alu.is_lt)

        z_sb = psb.tile([128, 2, BS], dt.float16, tag="zsb")
        fcw_sb = psb.tile([128, 2, HID], dt.float16, tag="fcw")
        fcb_sb = psb.tile([128, KC0], dt.float32, tag="fcb")
        for kc in range(2):
            nc.sync.dma_start(z_sb[:, kc, :], io["zT"][kc])
            nc.sync.dma_start(fcw_sb[:, kc, :], io["fcwT"][kc])
        for hc in range(KC0):
            nc.sync.dma_start(fcb_sb[:, hc:hc + 1], io["fcb"][hc])
        for hc in range(KC0):
            pfc = pps.tile([128, BS], dt.float32, tag="pfc")
            for kc in range(2):
                nc.tensor.matmul(pfc[:], fcw_sb[:, kc, hc * 128:(hc + 1) * 128],
                                 z_sb[:, kc, :], start=(kc == 0), stop=(kc == 1))
            nc.scalar.activation(fc_hT[:, hc, :], pfc[:], Relu,
                                 bias=fcb_sb[:, hc:hc + 1], scale=1.0)
        # gfcT[b, m*128+g] = (fc_h^T @ w_ih0^T): layer-0 time-constant term
        for m in range(MCH):
            pgf = pps.tile([4, 128], dt.float32, tag="pgf")
            for kc in range(KC0):
                nc.tensor.matmul(pgf[:], fc_hT[:, kc, :],
                                 wih[0][:, kc, m * 128:(m + 1) * 128],
                                 start=(kc == 0), stop=(kc == KC0 - 1))
            nc.vector.tensor_copy(gfcT[:, m, :], pgf[:])

    # ---- per-layer chunk loop ----------------------------------------
    full_cm = tc.For_i(0, RFULL, 1, name="rfull") if RFULL else None
    if full_cm is not None:
        full_cm.__enter__()
    passes = [0, 1, 2] + [1] * REPEAT + ([1] if RDEV else [])
    for pidx, l in enumerate(passes):
        in_dev_repeat = RDEV and pidx == len(passes) - 1
        rep_cm = tc.For_i(0, RDEV, 1, name="rep") if in_dev_repeat else None
        if rep_cm is not None:
            rep_cm.__enter__()
        kcl = KC0 if l == 0 else KC
        cur = hseq[l % 2]
        prev = hseq[(l - 1) % 2]
        nc.gpsimd.memset(hbf[:, TC - 1, :, :], 0.0)

        hint = (mybir.EngineType.PE,)
        with tc.For_i(0, T, TC, hint_engines=hint, name=f"pass{pidx}") as i:
            # -- chunk setup: gx GEMM + bias/fc seeding into PSUM --
            # pg banks: 0: r (j0..3 + pad), 1: z (j8..11 + pad),
            #           2: B=gx_n (j16..19) and C=gh_n (j20..23)
            pg = pgp.tile([128, 24, TC, BS], dt.float32, tag="pg")
            jm = lambda m: m if m < 4 else (m + 4 if m < 8 else m + 8)
            if l == 0:
                chd = stream.tile([128, TC, KC0, BS], dt.float16, tag="chd")
                for kc in range(KC0):
                    nc.sync.dma_start(chd[:, :, kc, :],
                                      io["chordT"][kc, :, bass.ds(i, TC), :])
                srcap = lambda kc: chd[:, :, kc, :]
            else:
                srcap = lambda kc: prev[:, bass.ds(i, TC), kc, :]

            for m in range(MCH):
                for kc in range(kcl):
                    # each bank cleared by its first MM: m in (0, 4, 8), kc 0
                    st = (kc == 0) and (m in (0, 4, 8))
                    nc.tensor.matmul(
                        pg[:, jm(m), :, :],
                        wih[l][:, kc, m * 128:(m + 1) * 128],
                        srcap(kc), start=st, stop=False,
                        skip_group_check=True)
            for j in list(range(0, 4)) + list(range(8, 12)) + list(range(16, 24)):
                nc.tensor.matmul(
                    pg[:, j, :, :], biasb[:, l, j, :], onehot[:],
                    start=False, stop=False, skip_group_check=True)
            if l == 0:
                for m in range(MCH):
                    nc.tensor.matmul(
                        pg[:, jm(m), :, :], gfcT[:, m, :], onehot[:],
                        start=False, stop=False, skip_group_check=True)

            if l == 2 and ABLATE != "nogates":
                mch = stream.tile([128, TC, 1, BS], dt.float16, tag="maskch")
                nc.sync.dma_start(mch[:, :, 0, :], mask[:, bass.ds(i, TC), :])
                osb = outp.tile([128, TC, KC, BS], dt.float32, tag="osb")

            # -- TC recurrence steps --
            for s in range(TC):
                sp = (s - 1) % TC
                if ABLATE == "nochain":
                    sp = TC - 1
                # r MMs (bank0) -> sigma_r; z MMs (bank1) -> sigma_z;
                # n MMs (bank2) overlap both sigmas
                rz = tmp.tile([128, 8, BS], dt.float32, tag="rz")
                for m in range(4):
                    for kc in range(KC):
                        nc.tensor.matmul(
                            pg[:, m, s, :],
                            whh[l][:, kc, m * 128:(m + 1) * 128],
                            hbf[:, sp, kc, :],
                            start=False, stop=(s == TC - 1 and kc == KC - 1),
                            skip_group_check=True)
                if ABLATE != "nogates":
                    nc.scalar.activation(rz[:, 0:4, :], pg[:, 0:4, s, :],
                                         Sigmoid)
                for m in range(4, 8):
                    for kc in range(KC):
                        nc.tensor.matmul(
                            pg[:, m + 4, s, :],
                            whh[l][:, kc, m * 128:(m + 1) * 128],
                            hbf[:, sp, kc, :],
                            start=False, stop=(s == TC - 1 and kc == KC - 1),
                            skip_group_check=True)
                if ABLATE != "nogates":
                    nc.scalar.activation(rz[:, 4:8, :], pg[:, 8:12, s, :],
                                         Sigmoid)
                for m in range(8, MCH):
                    for kc in range(KC):
                        nc.tensor.matmul(
                            pg[:, m + 12, s, :],
                            whh[l][:, kc, m * 128:(m + 1) * 128],
                            hbf[:, sp, kc, :],
                            start=False, stop=(s == TC - 1 and kc == KC - 1),
                            skip_group_check=True)
                if ABLATE == "nogates":
                    continue
                rn = tmp.tile([128, KC, BS], dt.float32, tag="rn")
                nc.vector.tensor_mul(rn[:], pg[:, 20:24, s, :], rz[:, 0:4, :])
                aN = tmp.tile([128, KC, BS], dt.float32, tag="aN")
                nc.vector.tensor_add(aN[:], rn[:], pg[:, 16:20, s, :])
                n = tmp.tile([128, KC, BS], dt.float32, tag="n")
                nc.scalar.activation(n[:], aN[:], Tanh)
                zc = tmp.tile([128, KC, BS], dt.float32, tag="zc")
                nc.vector.tensor_scalar(zc[:], rz[:, 4:8, :], -1.0, 1.0,
                                        op0=Alu.mult, op1=Alu.add)
                m2 = tmp.tile([128, KC, BS], dt.float32, tag="m2")
                nc.vector.tensor_mul(m2[:], rz[:, 4:8, :], hbf[:, sp, :, :])
                m1 = tmp.tile([128, KC, BS], dt.float32, tag="m1")
                nc.vector.tensor_mul(m1[:], n[:], zc[:])
                nc.vector.tensor_add(hbf[:, s, :, :], m1[:], m2[:])

            if l < 2:
                nc.sync.dma_start(cur[:, bass.ds(i, TC), :, :], hbf[:])
            elif ABLATE != "nogates":
                if True:
                    nc.vector.tensor_mul(
                        osb[:], hbf[:],
                        mch[:].broadcast_to([128, TC, KC, BS]))
                    nc.sync.dma_start(io["outT"][:, bass.ds(i, TC), :, :],
                                      osb[:])
        if rep_cm is not None:
            rep_cm.__exit__(None, None, None)
    if full_cm is not None:
        full_cm.__exit__(None, None, None)
    ctx.close()


_CACHE = {}


def _get_program():
    if "nc" not in _CACHE:
        _CACHE["nc"] = _build_program()
    return _CACHE["nc"]


def _prep_shared(fc_w, fc_b, ws):
    sh = {}
    sh["fcwT"] = np.ascontiguousarray(
        fc_w.T.reshape(2, 128, HID)).astype(BF)
    sh["fcb"] = np.ascontiguousarray(fc_b.reshape(KC0, 128, 1)).astype(F32)
    for l in range(3):
        w_ih, w_hh, _, _ = ws[l]
        kcl = KC0 if l == 0 else KC
        sh[f"wihT{l}"] = np.ascontiguousarray(
            w_ih.T.reshape(kcl, 128, G3)).astype(BF)
        sh[f"whhT{l}"] = np.ascontiguousarray(
            w_hh.T.reshape(KC, 128, G3)).astype(BF)
    bt = np.zeros((3, 24, 128), F32)
    for l in range(3):
        _, _, b_ih, b_hh = ws[l]
        bi = b_ih.reshape(MCH, 128)
        bh = b_hh.reshape(MCH, 128)
        bt[l, 0:4] = bi[0:4] + bh[0:4]
        bt[l, 8:12] = bi[4:8] + bh[4:8]
        bt[l, 16:20] = bi[8:12]
        bt[l, 20:24] = bh[8:12]
    sh["biasT"] = np.broadcast_to(
        bt.reshape(1, -1), (4, 3 * 24 * 128)).astype(BF).copy()
    oh = np.zeros((4, TC, BS), F32)
    for k in range(BS):
        oh[k, :, k] = 1.0
    sh["onehotT"] = oh.reshape(4, TC * BS).astype(BF)
    sh["iota"] = np.broadcast_to(
        np.arange(T, dtype=F32)[None, :], (128, T)).copy()
    return sh


def kernel(z, seq_lens, chord_embedding, fc_w, fc_b,
           w_ih0, w_hh0, b_ih0, b_hh0,
           w_ih1, w_hh1, b_ih1, b_hh1,
           w_ih2, w_hh2, b_ih2, b_hh2):
    z = np.asarray(z, F32)
    chord = np.asarray(chord_embedding, F32)
    seq = np.asarray(seq_lens)
    ws = [(np.asarray(w_ih0, F32), np.asarray(w_hh0, F32),
           np.asarray(b_ih0, F32), np.asarray(b_hh0, F32)),
          (np.asarray(w_ih1, F32), np.asarray(w_hh1, F32),
           np.asarray(b_ih1, F32), np.asarray(b_hh1, F32)),
          (np.asarray(w_ih2, F32), np.asarray(w_hh2, F32),
           np.asarray(b_ih2, F32), np.asarray(b_hh2, F32))]

    in_maps = _make_in_maps(z, seq, chord, np.asarray(fc_w, F32),
                            np.asarray(fc_b, F32), ws)
    res = _execute(in_maps)
    return _assemble(res.results)


def _make_in_maps(z, seq, chord, fc_w, fc_b, ws):
    sh = _prep_shared(fc_w, fc_b, ws)
    in_maps = []
    for c in range(NCORES):
        bs = slice(c * BS, (c + 1) * BS)
        m = dict(sh)
        m["chordT"] = np.ascontiguousarray(
            (chord[bs].transpose(2, 1, 0) / 100.0)
            .reshape(KC0, 128, T, BS)).astype(BF)
        m["zT"] = np.ascontiguousarray(
            z[bs].T.reshape(2, 128, BS)).astype(BF)
        m["seqrep"] = np.broadcast_to(
            seq[bs].astype(F32)[None, :], (128, BS)).copy()
        in_maps.append(m)
    return in_maps


def _execute(in_maps, **kw):
    nc = _get_program()
    return bass_utils.run_bass_kernel_spmd(nc, in_maps, list(range(NCORES)), **kw)


def _assemble(results):
    out = np.empty((B, T, H), F32)
    for c in range(NCORES):
        outT = np.asarray(results[c]["outT"])       # [128,T,KC,BS]
        out[c * BS:(c + 1) * BS] = (
            outT.transpose(3, 1, 2, 0).reshape(BS, T, H))
    return out


# revision 18
# speedup vs baseline: 1.0179x; 1.0179x over previous
"""Trainium2 Bass kernel for nn_Decoder (fc + 3-layer GRU + mask).

Strategy: data-parallel over batch B=32 across 8 cores (4 samples/core).
Gate-major layout: gate/hidden dims on partitions, (hc, b) on free.

v2: the per-chunk input projections (gx), gate biases, and the layer-0
fc term are all accumulated INTO PSUM by the tensor engine, so the
per-step gate chain reads pre-activations straight from PSUM:

  PSUM pg [128, 16, TC, BS] per chunk (2 banks):
    bank0 j=0..7  : gx_rz + (b_ih+b_hh)_rz + gh_rz   (step MMs accumulate)
    bank1 j=8..11 : gx_n + b_ih_n  (read-only per step)
    bank1 j=12..15: b_hh_n + gh_n                     (step MMs accumulate)

Per step: PE 32 rz-MMs -> sigma (ACT, overlaps the 16 n-MMs) ->
rn = C*r -> aN = rn + B -> tanh -> (1-z), z*h_prev under tanh's shadow
-> h = n*(1-z) + z*h_prev written directly to the fp16 step buffer.
"""

import os
import sys
from contextlib import ExitStack

for _p in ("/opt/trn_rl_repo",):
    if _p not in sys.path:
        sys.path.insert(0, _p)

import numpy as np
import ml_dtypes

import concourse.bass as bass
import concourse.bacc as bacc
import concourse.mybir as mybir
import concourse.tile as tile
from concourse import bass_utils

BF = np.float16
F32 = np.float32
F8 = ml_dtypes.float8_e4m3
dt = mybir.dt

NCORES = 8
B, T = 32, 512
BS = B // NCORES          # 4 samples per core
TC = 16                   # time-steps per chunk
NCHUNK = T // TC          # 32
HID = 1024                # layer-0 input dim
H = 512                   # GRU hidden
G3 = 3 * H                # 1536
MCH = G3 // 128           # 12 gate-dim chunks: r0..3 z0..3 n0..3
KC = H // 128             # 4 hidden chunks
KC0 = HID // 128          # 8 input chunks for layer 0
FREE = KC * BS            # 16
REPEAT = 0
ABLATE = ""               # "nochain", "nogates"
RDEV = 0                  # device-side repeats of a layer-1 pass
RFULL = 0                 # device-side repeats of the full 3-pass body

Sigmoid = mybir.ActivationFunctionType.Sigmoid
Tanh = mybir.ActivationFunctionType.Tanh
Relu = mybir.ActivationFunctionType.Relu
Alu = mybir.AluOpType


def _declare_io(nc):
    d = {}
    inp = lambda n, s, t: nc.dram_tensor(n, s, t, kind="ExternalInput").ap()
    d["chordT"] = inp("chordT", [KC0, 128, T, BS], dt.float16)
    d["zT"] = inp("zT", [2, 128, BS], dt.float16)
    d["fcwT"] = inp("fcwT", [2, 128, HID], dt.float16)
    d["fcb"] = inp("fcb", [KC0, 128, 1], dt.float32)
    d["wihT0"] = inp("wihT0", [KC0, 128, G3], dt.float16)
    d["wihT1"] = inp("wihT1", [KC, 128, G3], dt.float16)
    d["wihT2"] = inp("wihT2", [KC, 128, G3], dt.float16)
    d["whhT0"] = inp("whhT0", [KC, 128, G3], dt.float16)
    d["whhT1"] = inp("whhT1", [KC, 128, G3], dt.float16)
    d["whhT2"] = inp("whhT2", [KC, 128, G3], dt.float16)
    d["biasT"] = inp("biasT", [4, 3 * 24 * 128], dt.float16)
    d["onehotT"] = inp("onehotT", [4, TC * BS], dt.float16)
    d["iota"] = inp("iota", [128, T], dt.float32)
    d["seqrep"] = inp("seqrep", [128, BS], dt.float32)
    d["outT"] = nc.dram_tensor("outT", [128, T, KC, BS], dt.float32,
                               kind="ExternalOutput").ap()
    return d


def _build_program(debug=False):
    nc = bacc.Bacc("TRN2", target_bir_lowering=False, debug=debug,
                   num_devices=NCORES)
    io = _declare_io(nc)
    with tile.TileContext(nc) as tc:
        _emit(tc, io)
    nc.compile()
    return nc


def _emit(tc, io):
    nc = tc.nc
    ctx = ExitStack()
    const = ctx.enter_context(tc.tile_pool(name="const", bufs=1))
    stream = ctx.enter_context(tc.tile_pool(name="stream", bufs=3))
    tmp = ctx.enter_context(tc.tile_pool(name="tmp", bufs=3))
    outp = ctx.enter_context(tc.tile_pool(name="outp", bufs=2))
    pgp = ctx.enter_context(tc.tile_pool(name="pg", bufs=2, space="PSUM"))

    # ---- persistent SBUF tensors -------------------------------------
    wih = [const.tile([128, KC0, G3], dt.float16, tag="wih0", name="wih0"),
           const.tile([128, KC, G3], dt.float16, tag="wih1", name="wih1"),
           const.tile([128, KC, G3], dt.float16, tag="wih2", name="wih2")]
    whh = [const.tile([128, KC, G3], dt.float16, tag=f"whh{l}",
                      name=f"whh{l}") for l in range(3)]
    hseq = [const.tile([128, T, KC, BS], dt.float16, tag=f"hseq{i}",
                       name=f"hseq{i}") for i in range(2)]
    mask = const.tile([128, T, BS], dt.float16, tag="mask")
    biasb = const.tile([4, 3, 24, 128], dt.float16, tag="biasb")
    onehot = const.tile([4, TC * BS], dt.float16, tag="onehot")
    fc_hT = const.tile([128, KC0, BS], dt.float16, tag="fchT")
    gfcT = const.tile([4, MCH, 128], dt.float16, tag="gfcT")
    hbf = const.tile([128, TC, KC, BS], dt.float16, tag="hbf")

    # ---- load weights / constants ------------------------------------
    for kc in range(KC0):
        nc.sync.dma_start(wih[0][:, kc, :], io["wihT0"][kc])
    for l in (1, 2):
        for kc in range(KC):
            nc.sync.dma_start(wih[l][:, kc, :], io[f"wihT{l}"][kc])
    for l in range(3):
        for kc in range(KC):
            nc.sync.dma_start(whh[l][:, kc, :], io[f"whhT{l}"][kc])
    nc.sync.dma_start(
        biasb[:].rearrange("p l j g -> p (l j g)"), io["biasT"])
    nc.sync.dma_start(onehot[:], io["onehotT"])

    # ---- prologue: mask, fc, gfcT ------------------------------------
    with ExitStack() as pctx:
        psb = pctx.enter_context(tc.tile_pool(name="psb", bufs=2))
        pps = pctx.enter_context(tc.tile_pool(name="pps", bufs=1,
                                              space="PSUM"))

        iota_sb = psb.tile([128, T], dt.float32, tag="iota")
        seq_sb = psb.tile([128, BS], dt.float32, tag="seq")
        nc.sync.dma_start(iota_sb[:], io["iota"])
        nc.sync.dma_start(seq_sb[:], io["seqrep"])
        for b in range(BS):
            nc.vector.tensor_scalar(mask[:, :, b], iota_sb[:],
                                    seq_sb[:, b:b + 1], None, op0=Alu.is_lt)

        z_sb = psb.tile([128, 2, BS], dt.float16, tag="zsb")
        fcw_sb = psb.tile([128, 2, HID], dt.float16, tag="fcw")
        fcb_sb = psb.tile([128, KC0], dt.float32, tag="fcb")
        for kc in range(2):
            nc.sync.dma_start(z_sb[:, kc, :], io["zT"][kc])
            nc.sync.dma_start(fcw_sb[:, kc, :], io["fcwT"][kc])
        for hc in range(KC0):
            nc.sync.dma_start(fcb_sb[:, hc:hc + 1], io["fcb"][hc])
        for hc in range(KC0):
            pfc = pps.tile([128, BS], dt.float32, tag="pfc")
            for kc in range(2):
                nc.tensor.matmul(pfc[:], fcw_sb[:, kc, hc * 128:(hc + 1) * 128],
                                 z_sb[:, kc, :], start=(kc == 0), stop=(kc == 1))
            nc.scalar.activation(fc_hT[:, hc, :], pfc[:], Relu,
                                 bias=fcb_sb[:, hc:hc + 1], scale=1.0)
        # gfcT[b, m*128+g] = (fc_h^T @ w_ih0^T): layer-0 time-constant term
        for m in range(MCH):
            pgf = pps.tile([4, 128], dt.float32, tag="pgf")
            for kc in range(KC0):
                nc.tensor.matmul(pgf[:], fc_hT[:, kc, :],
                                 wih[0][:, kc, m * 128:(m + 1) * 128],
                                 start=(kc == 0), stop=(kc == KC0 - 1))
            nc.vector.tensor_copy(gfcT[:, m, :], pgf[:])

    # ---- per-layer chunk loop ----------------------------------------
    full_cm = tc.For_i(0, RFULL, 1, name="rfull") if RFULL else None
    if full_cm is not None:
        full_cm.__enter__()
    passes = [0, 1, 2] + [1] * REPEAT + ([1] if RDEV else [])
    for pidx, l in enumerate(passes):
        in_dev_repeat = RDEV and pidx == len(passes) - 1
        rep_cm = tc.For_i(0, RDEV, 1, name="rep") if in_dev_repeat else None
        if rep_cm is not None:
            rep_cm.__enter__()
        kcl = KC0 if l == 0 else KC
        cur = hseq[l % 2]
        prev = hseq[(l - 1) % 2]
        nc.gpsimd.memset(hbf[:, TC - 1, :, :], 0.0)

        hint = (mybir.EngineType.PE,)
        with tc.For_i(0, T, TC, hint_engines=hint, name=f"pass{pidx}") as i:
            # -- chunk setup: gx GEMM + bias/fc seeding into PSUM --
            # pg banks: 0: r (j0..3 + pad), 1: z (j8..11 + pad),
            #           2: B=gx_n (j16..19) and C=gh_n (j20..23)
            pg = pgp.tile([128, 24, TC, BS], dt.float32, tag="pg")
            jm = lambda m: m if m < 4 else (m + 4 if m < 8 else m + 8)
            if l == 0:
                chd = stream.tile([128, TC, KC0, BS], dt.float16, tag="chd")
                for kc in range(KC0):
                    nc.sync.dma_start(chd[:, :, kc, :],
                                      io["chordT"][kc, :, bass.ds(i, TC), :])
                srcap = lambda kc: chd[:, :, kc, :]
            else:
                srcap = lambda kc: prev[:, bass.ds(i, TC), kc, :]

            for m in range(MCH):
                for kc in range(kcl):
                    # each bank cleared by its first MM: m in (0, 4, 8), kc 0
                    st = (kc == 0) and (m in (0, 4, 8))
                    nc.tensor.matmul(
                        pg[:, jm(m), :, :],
                        wih[l][:, kc, m * 128:(m + 1) * 128],
                        srcap(kc), start=st, stop=False,
                        skip_group_check=True)
            for j in list(range(0, 4)) + list(range(8, 12)) + list(range(16, 24)):
                nc.tensor.matmul(
                    pg[:, j, :, :], biasb[:, l, j, :], onehot[:],
                    start=False, stop=False, skip_group_check=True)
            if l == 0:
                for m in range(MCH):
                    nc.tensor.matmul(
                        pg[:, jm(m), :, :], gfcT[:, m, :], onehot[:],
                        start=False, stop=False, skip_group_check=True)

            if l == 2 and ABLATE != "nogates":
                mch = stream.tile([128, TC, 1, BS], dt.float16, tag="maskch")
                nc.sync.dma_start(mch[:, :, 0, :], mask[:, bass.ds(i, TC), :])
                osb = outp.tile([128, TC, KC, BS], dt.float32, tag="osb")

            # -- TC recurrence steps --
            for s in range(TC):
                sp = (s - 1) % TC
                if ABLATE == "nochain":
                    sp = TC - 1
                # r MMs (bank0) -> sigma_r; z MMs (bank1) -> sigma_z;
                # n MMs (bank2) overlap both sigmas
                kcs = (0, 1) if ABLATE == "halfk" else range(KC)
                rz = tmp.tile([128, 8, BS], dt.float32, tag="rz")
                # PE order: r-MMs, n-MMs, z-MMs. sigma_r fires after the
                # r block; rn/aN/tanh/d hide under the z block; only
                # sigma_z -> zd -> h trail the PE.  h = n + z*(h_prev - n)
                for m in range(4):
                    for kc in kcs:
                        nc.tensor.matmul(
                            pg[:, m, s, :],
                            whh[l][:, kc, m * 128:(m + 1) * 128],
                            hbf[:, sp, kc, :],
                            start=False, stop=(s == TC - 1 and kc == max(kcs)),
                            skip_group_check=True)
                if ABLATE != "nogates":
                    nc.scalar.activation(rz[:, 0:4, :], pg[:, 0:4, s, :],
                                         Sigmoid)
                for m in range(8, MCH):
                    for kc in kcs:
                        nc.tensor.matmul(
                            pg[:, m + 12, s, :],
                            whh[l][:, kc, m * 128:(m + 1) * 128],
                            hbf[:, sp, kc, :],
                            start=False, stop=(s == TC - 1 and kc == max(kcs)),
                            skip_group_check=True)
                if ABLATE != "nogates":
                    rn = tmp.tile([128, KC, BS], dt.float32, tag="rn")
                    nc.vector.tensor_mul(rn[:], pg[:, 20:24, s, :],
                                         rz[:, 0:4, :])
                    aN = tmp.tile([128, KC, BS], dt.float32, tag="aN")
                    nc.vector.tensor_add(aN[:], rn[:], pg[:, 16:20, s, :])
                    n = tmp.tile([128, KC, BS], dt.float32, tag="n")
                    nc.scalar.activation(n[:], aN[:], Tanh)
                for m in range(4, 8):
                    for kc in kcs:
                        nc.tensor.matmul(
                            pg[:, m + 4, s, :],
                            whh[l][:, kc, m * 128:(m + 1) * 128],
                            hbf[:, sp, kc, :],
                            start=False, stop=(s == TC - 1 and kc == max(kcs)),
                            skip_group_check=True)
                if ABLATE == "nogates":
                    continue
                nc.scalar.activation(rz[:, 4:8, :], pg[:, 8:12, s, :],
                                     Sigmoid)
                d = tmp.tile([128, KC, BS], dt.float32, tag="d")
                nc.vector.tensor_sub(d[:], hbf[:, sp, :, :], n[:])
                zd = tmp.tile([128, KC, BS], dt.float32, tag="zd")
                nc.vector.tensor_mul(zd[:], rz[:, 4:8, :], d[:])
                nc.vector.tensor_add(hbf[:, s, :, :], n[:], zd[:])

            if l < 2:
                if ABLATE != "nodma":
                    nc.sync.dma_start(cur[:, bass.ds(i, TC), :, :], hbf[:])
                else:
                    nc.gpsimd.memset(cur[:, 0, 0, :], 0.0)
            elif ABLATE not in ("nogates", "nodma"):
                if True:
                    nc.vector.tensor_mul(
                        osb[:], hbf[:],
                        mch[:].broadcast_to([128, TC, KC, BS]))
                    nc.sync.dma_start(io["outT"][:, bass.ds(i, TC), :, :],
                                      osb[:])
        if rep_cm is not None:
            rep_cm.__exit__(None, None, None)
    if full_cm is not None:
        full_cm.__exit__(None, None, None)
    ctx.close()


_CACHE = {}


def _get_program():
    if "nc" not in _CACHE:
        _CACHE["nc"] = _build_program()
    return _CACHE["nc"]


def _prep_shared(fc_w, fc_b, ws):
    sh = {}
    sh["fcwT"] = np.ascontiguousarray(
        fc_w.T.reshape(2, 128, HID)).astype(BF)
    sh["fcb"] = np.ascontiguousarray(fc_b.reshape(KC0, 128, 1)).astype(F32)
    for l in range(3):
        w_ih, w_hh, _, _ = ws[l]
        kcl = KC0 if l == 0 else KC
        sh[f"wihT{l}"] = np.ascontiguousarray(
            w_ih.T.reshape(kcl, 128, G3)).astype(BF)
        sh[f"whhT{l}"] = np.ascontiguousarray(
            w_hh.T.reshape(KC, 128, G3)).astype(BF)
    bt = np.zeros((3, 24, 128), F32)
    for l in range(3):
        _, _, b_ih, b_hh = ws[l]
        bi = b_ih.reshape(MCH, 128)
        bh = b_hh.reshape(MCH, 128)
        bt[l, 0:4] = bi[0:4] + bh[0:4]
        bt[l, 8:12] = bi[4:8] + bh[4:8]
        bt[l, 16:20] = bi[8:12]
        bt[l, 20:24] = bh[8:12]
    sh["biasT"] = np.broadcast_to(
        bt.reshape(1, -1), (4, 3 * 24 * 128)).astype(BF).copy()
    oh = np.zeros((4, TC, BS), F32)
    for k in range(BS):
        oh[k, :, k] = 1.0
    sh["onehotT"] = oh.reshape(4, TC * BS).astype(BF)
    sh["iota"] = np.broadcast_to(
        np.arange(T, dtype=F32)[None, :], (128, T)).copy()
    return sh


def kernel(z, seq_lens, chord_embedding, fc_w, fc_b,
           w_ih0, w_hh0, b_ih0, b_hh0,
           w_ih1, w_hh1, b_ih1, b_hh1,
           w_ih2, w_hh2, b_ih2, b_hh2):
    z = np.asarray(z, F32)
    chord = np.asarray(chord_embedding, F32)
    seq = np.asarray(seq_lens)
    ws = [(np.asarray(w_ih0, F32), np.asarray(w_hh0, F32),
           np.asarray(b_ih0, F32), np.asarray(b_hh0, F32)),
          (np.asarray(w_ih1, F32), np.asarray(w_hh1, F32),
           np.asarray(b_ih1, F32), np.asarray(b_hh1, F32)),
          (np.asarray(w_ih2, F32), np.asarray(w_hh2, F32),
           np.asarray(b_ih2, F32), np.asarray(b_hh2, F32))]

    in_maps = _make_in_maps(z, seq, chord, np.asarray(fc_w, F32),
                            np.asarray(fc_b, F32), ws)
    res = _execute(in_maps)
    return _assemble(res.results)


def _make_in_maps(z, seq, chord, fc_w, fc_b, ws):
    sh = _prep_shared(fc_w, fc_b, ws)
    in_maps = []
    for c in range(NCORES):
        bs = slice(c * BS, (c + 1) * BS)
        m = dict(sh)
        m["chordT"] = np.ascontiguousarray(
            (chord[bs].transpose(2, 1, 0) / 100.0)
            .reshape(KC0, 128, T, BS)).astype(BF)
        m["zT"] = np.ascontiguousarray(
            z[bs].T.reshape(2, 128, BS)).astype(BF)
        m["seqrep"] = np.broadcast_to(
            seq[bs].astype(F32)[None, :], (128, BS)).copy()
        in_maps.append(m)
    return in_maps


def _execute(in_maps, **kw):
    nc = _get_program()
    return bass_utils.run_bass_kernel_spmd(nc, in_maps, list(range(NCORES)), **kw)


def _assemble(results):
    out = np.empty((B, T, H), F32)
    for c in range(NCORES):
        outT = np.asarray(results[c]["outT"])       # [128,T,KC,BS]
        out[c * BS:(c + 1) * BS] = (
            outT.transpose(3, 1, 2, 0).reshape(BS, T, H))
    return out


# revision 19
# speedup vs baseline: 1.0700x; 1.0513x over previous
"""Trainium2 Bass kernel for nn_Decoder (fc + 3-layer GRU + mask).

Strategy: data-parallel over batch B=32 across 8 cores (4 samples/core).
Gate-major layout: gate/hidden dims on partitions, (hc, b) on free.

v2: the per-chunk input projections (gx), gate biases, and the layer-0
fc term are all accumulated INTO PSUM by the tensor engine, so the
per-step gate chain reads pre-activations straight from PSUM:

  PSUM pg [128, 16, TC, BS] per chunk (2 banks):
    bank0 j=0..7  : gx_rz + (b_ih+b_hh)_rz + gh_rz   (step MMs accumulate)
    bank1 j=8..11 : gx_n + b_ih_n  (read-only per step)
    bank1 j=12..15: b_hh_n + gh_n                     (step MMs accumulate)

Per step: PE 32 rz-MMs -> sigma (ACT, overlaps the 16 n-MMs) ->
rn = C*r -> aN = rn + B -> tanh -> (1-z), z*h_prev under tanh's shadow
-> h = n*(1-z) + z*h_prev written directly to the fp16 step buffer.
"""

import os
import sys
from contextlib import ExitStack

for _p in ("/opt/trn_rl_repo",):
    if _p not in sys.path:
        sys.path.insert(0, _p)

import numpy as np
import ml_dtypes

import concourse.bass as bass
import concourse.bacc as bacc
import concourse.mybir as mybir
import concourse.tile as tile
from concourse import bass_utils

BF = np.float16
F32 = np.float32
F8 = ml_dtypes.float8_e4m3
dt = mybir.dt

NCORES = 8
B, T = 32, 512
BS = B // NCORES          # 4 samples per core
TC = 32                   # time-steps per chunk
NCHUNK = T // TC          # 32
HID = 1024                # layer-0 input dim
H = 512                   # GRU hidden
G3 = 3 * H                # 1536
MCH = G3 // 128           # 12 gate-dim chunks: r0..3 z0..3 n0..3
KC = H // 128             # 4 hidden chunks
KC0 = HID // 128          # 8 input chunks for layer 0
FREE = KC * BS            # 16
REPEAT = 0
ABLATE = ""               # "nochain", "nogates"
RDEV = 0                  # device-side repeats of a layer-1 pass
RFULL = 0                 # device-side repeats of the full 3-pass body

Sigmoid = mybir.ActivationFunctionType.Sigmoid
Tanh = mybir.ActivationFunctionType.Tanh
Relu = mybir.ActivationFunctionType.Relu
Alu = mybir.AluOpType


def _declare_io(nc):
    d = {}
    inp = lambda n, s, t: nc.dram_tensor(n, s, t, kind="ExternalInput").ap()
    d["chordT"] = inp("chordT", [KC0, 128, T, BS], dt.float16)
    d["zT"] = inp("zT", [2, 128, BS], dt.float16)
    d["fcwT"] = inp("fcwT", [2, 128, HID], dt.float16)
    d["fcb"] = inp("fcb", [KC0, 128, 1], dt.float32)
    d["wihT0"] = inp("wihT0", [KC0, 128, G3], dt.float16)
    d["wihT1"] = inp("wihT1", [KC, 128, G3], dt.float16)
    d["wihT2"] = inp("wihT2", [KC, 128, G3], dt.float16)
    d["whhT0"] = inp("whhT0", [KC, 128, G3], dt.float16)
    d["whhT1"] = inp("whhT1", [KC, 128, G3], dt.float16)
    d["whhT2"] = inp("whhT2", [KC, 128, G3], dt.float16)
    d["biasT"] = inp("biasT", [4, 3 * 16 * 128], dt.float16)
    d["onehotT"] = inp("onehotT", [4, TC * BS], dt.float16)
    d["iota"] = inp("iota", [128, T], dt.float32)
    d["seqrep"] = inp("seqrep", [128, BS], dt.float32)
    d["outT"] = nc.dram_tensor("outT", [128, T, KC, BS], dt.float32,
                               kind="ExternalOutput").ap()
    return d


def _build_program(debug=False):
    nc = bacc.Bacc("TRN2", target_bir_lowering=False, debug=debug,
                   num_devices=NCORES)
    io = _declare_io(nc)
    with tile.TileContext(nc) as tc:
        _emit(tc, io)
    nc.compile()
    return nc


def _emit(tc, io):
    nc = tc.nc
    ctx = ExitStack()
    const = ctx.enter_context(tc.tile_pool(name="const", bufs=1))
    stream = ctx.enter_context(tc.tile_pool(name="stream", bufs=3))
    tmp = ctx.enter_context(tc.tile_pool(name="tmp", bufs=3))
    outp = ctx.enter_context(tc.tile_pool(name="outp", bufs=2))
    pgp = ctx.enter_context(tc.tile_pool(name="pg", bufs=1, space="PSUM"))

    # ---- persistent SBUF tensors -------------------------------------
    wih = [const.tile([128, KC0, G3], dt.float16, tag="wih0", name="wih0"),
           const.tile([128, KC, G3], dt.float16, tag="wih1", name="wih1"),
           const.tile([128, KC, G3], dt.float16, tag="wih2", name="wih2")]
    whh = [const.tile([128, KC, G3], dt.float16, tag=f"whh{l}",
                      name=f"whh{l}") for l in range(3)]
    hseq = [const.tile([128, T, KC, BS], dt.float16, tag=f"hseq{i}",
                       name=f"hseq{i}") for i in range(2)]
    mask = const.tile([128, T, BS], dt.float16, tag="mask")
    biasb = const.tile([4, 3, 16, 128], dt.float16, tag="biasb")
    onehot = const.tile([4, TC * BS], dt.float16, tag="onehot")
    fc_hT = const.tile([128, KC0, BS], dt.float16, tag="fchT")
    gfcT = const.tile([4, MCH, 128], dt.float16, tag="gfcT")
    hbf = const.tile([128, TC, KC, BS], dt.float16, tag="hbf")

    # ---- load weights / constants ------------------------------------
    for kc in range(KC0):
        nc.sync.dma_start(wih[0][:, kc, :], io["wihT0"][kc])
    for l in (1, 2):
        for kc in range(KC):
            nc.sync.dma_start(wih[l][:, kc, :], io[f"wihT{l}"][kc])
    for l in range(3):
        for kc in range(KC):
            nc.sync.dma_start(whh[l][:, kc, :], io[f"whhT{l}"][kc])
    nc.sync.dma_start(
        biasb[:].rearrange("p l j g -> p (l j g)"), io["biasT"])
    nc.sync.dma_start(onehot[:], io["onehotT"])

    # ---- prologue: mask, fc, gfcT ------------------------------------
    with ExitStack() as pctx:
        psb = pctx.enter_context(tc.tile_pool(name="psb", bufs=2))
        pps = pctx.enter_context(tc.tile_pool(name="pps", bufs=1,
                                              space="PSUM"))

        iota_sb = psb.tile([128, T], dt.float32, tag="iota")
        seq_sb = psb.tile([128, BS], dt.float32, tag="seq")
        nc.sync.dma_start(iota_sb[:], io["iota"])
        nc.sync.dma_start(seq_sb[:], io["seqrep"])
        for b in range(BS):
            nc.vector.tensor_scalar(mask[:, :, b], iota_sb[:],
                                    seq_sb[:, b:b + 1], None, op0=Alu.is_lt)

        z_sb = psb.tile([128, 2, BS], dt.float16, tag="zsb")
        fcw_sb = psb.tile([128, 2, HID], dt.float16, tag="fcw")
        fcb_sb = psb.tile([128, KC0], dt.float32, tag="fcb")
        for kc in range(2):
            nc.sync.dma_start(z_sb[:, kc, :], io["zT"][kc])
            nc.sync.dma_start(fcw_sb[:, kc, :], io["fcwT"][kc])
        for hc in range(KC0):
            nc.sync.dma_start(fcb_sb[:, hc:hc + 1], io["fcb"][hc])
        for hc in range(KC0):
            pfc = pps.tile([128, BS], dt.float32, tag="pfc")
            for kc in range(2):
                nc.tensor.matmul(pfc[:], fcw_sb[:, kc, hc * 128:(hc + 1) * 128],
                                 z_sb[:, kc, :], start=(kc == 0), stop=(kc == 1))
            nc.scalar.activation(fc_hT[:, hc, :], pfc[:], Relu,
                                 bias=fcb_sb[:, hc:hc + 1], scale=1.0)
        # gfcT[b, m*128+g] = (fc_h^T @ w_ih0^T): layer-0 time-constant term
        for m in range(MCH):
            pgf = pps.tile([4, 128], dt.float32, tag="pgf")
            for kc in range(KC0):
                nc.tensor.matmul(pgf[:], fc_hT[:, kc, :],
                                 wih[0][:, kc, m * 128:(m + 1) * 128],
                                 start=(kc == 0), stop=(kc == KC0 - 1))
            nc.vector.tensor_copy(gfcT[:, m, :], pgf[:])

    # ---- per-layer chunk loop ----------------------------------------
    full_cm = tc.For_i(0, RFULL, 1, name="rfull") if RFULL else None
    if full_cm is not None:
        full_cm.__enter__()
    passes = [0, 1, 2] + [1] * REPEAT + ([1] if RDEV else [])
    for pidx, l in enumerate(passes):
        in_dev_repeat = RDEV and pidx == len(passes) - 1
        rep_cm = tc.For_i(0, RDEV, 1, name="rep") if in_dev_repeat else None
        if rep_cm is not None:
            rep_cm.__enter__()
        kcl = KC0 if l == 0 else KC
        cur = hseq[l % 2]
        prev = hseq[(l - 1) % 2]
        nc.gpsimd.memset(hbf[:, TC - 1, :, :], 0.0)

        hint = (mybir.EngineType.PE,)
        with tc.For_i(0, T, TC, hint_engines=hint, name=f"pass{pidx}") as i:
            # -- chunk setup: gx GEMM + bias/fc seeding into PSUM --
            # pg banks (TC=32, 128 f32 per j-slot): 0: r j0..3,
            # 1: z j4..7, 2: B=gx_n j8..11, 3: C=gh_n j12..15
            pg = pgp.tile([128, 16, TC, BS], dt.float32, tag="pg")
            jm = lambda m: m
            if l == 0:
                chd = stream.tile([128, TC, KC0, BS], dt.float16, tag="chd")
                for kc in range(KC0):
                    nc.sync.dma_start(chd[:, :, kc, :],
                                      io["chordT"][kc, :, bass.ds(i, TC), :])
                srcap = lambda kc: chd[:, :, kc, :]
            else:
                srcap = lambda kc: prev[:, bass.ds(i, TC), kc, :]

            for m in range(MCH):
                for kc in range(kcl):
                    # banks 0-2 cleared by first MM: m in (0, 4, 8), kc 0
                    st = (kc == 0) and (m in (0, 4, 8))
                    nc.tensor.matmul(
                        pg[:, jm(m), :, :],
                        wih[l][:, kc, m * 128:(m + 1) * 128],
                        srcap(kc), start=st, stop=False,
                        skip_group_check=True)
            for j in range(16):
                # j=12 is bank3's first writer: start=True clears the bank
                nc.tensor.matmul(
                    pg[:, j, :, :], biasb[:, l, j, :], onehot[:],
                    start=(j == 12), stop=False, skip_group_check=True)
            if l == 0:
                for m in range(MCH):
                    nc.tensor.matmul(
                        pg[:, jm(m), :, :], gfcT[:, m, :], onehot[:],
                        start=False, stop=False, skip_group_check=True)

            if l == 2 and ABLATE != "nogates":
                mch = stream.tile([128, TC, 1, BS], dt.float16, tag="maskch")
                nc.sync.dma_start(mch[:, :, 0, :], mask[:, bass.ds(i, TC), :])
                osb = outp.tile([128, TC, KC, BS], dt.float32, tag="osb")

            # -- TC recurrence steps --
            for s in range(TC):
                sp = (s - 1) % TC
                if ABLATE == "nochain":
                    sp = TC - 1
                # r MMs (bank0) -> sigma_r; z MMs (bank1) -> sigma_z;
                # n MMs (bank2) overlap both sigmas
                kcs = (0, 1) if ABLATE == "halfk" else range(KC)
                rz = tmp.tile([128, 8, BS], dt.float32, tag="rz")
                # PE order: r-MMs, n-MMs, z-MMs. sigma_r fires after the
                # r block; rn/aN/tanh/d hide under the z block; only
                # sigma_z -> zd -> h trail the PE.  h = n + z*(h_prev - n)
                for m in range(4):
                    for kc in kcs:
                        nc.tensor.matmul(
                            pg[:, m, s, :],
                            whh[l][:, kc, m * 128:(m + 1) * 128],
                            hbf[:, sp, kc, :],
                            start=False, stop=(s == TC - 1 and kc == max(kcs)),
                            skip_group_check=True)
                if ABLATE != "nogates":
                    nc.scalar.activation(rz[:, 0:4, :], pg[:, 0:4, s, :],
                                         Sigmoid)
                for m in range(8, MCH):
                    for kc in kcs:
                        nc.tensor.matmul(
                            pg[:, m + 4, s, :],
                            whh[l][:, kc, m * 128:(m + 1) * 128],
                            hbf[:, sp, kc, :],
                            start=False, stop=(s == TC - 1 and kc == max(kcs)),
                            skip_group_check=True)
                if ABLATE != "nogates":
                    rn = tmp.tile([128, KC, BS], dt.float32, tag="rn")
                    nc.vector.tensor_mul(rn[:], pg[:, 12:16, s, :],
                                         rz[:, 0:4, :])
                    aN = tmp.tile([128, KC, BS], dt.float32, tag="aN")
                    nc.vector.tensor_add(aN[:], rn[:], pg[:, 8:12, s, :])
                    n = tmp.tile([128, KC, BS], dt.float32, tag="n")
                    nc.scalar.activation(n[:], aN[:], Tanh)
                for m in range(4, 8):
                    for kc in kcs:
                        nc.tensor.matmul(
                            pg[:, m, s, :],
                            whh[l][:, kc, m * 128:(m + 1) * 128],
                            hbf[:, sp, kc, :],
                            start=False, stop=(s == TC - 1 and kc == max(kcs)),
                            skip_group_check=True)
                if ABLATE == "nogates":
                    continue
                nc.scalar.activation(rz[:, 4:8, :], pg[:, 4:8, s, :],
                                     Sigmoid)
                d = tmp.tile([128, KC, BS], dt.float32, tag="d")
                nc.vector.tensor_sub(d[:], hbf[:, sp, :, :], n[:])
                zd = tmp.tile([128, KC, BS], dt.float32, tag="zd")
                nc.vector.tensor_mul(zd[:], rz[:, 4:8, :], d[:])
                nc.vector.tensor_add(hbf[:, s, :, :], n[:], zd[:])

            if l < 2:
                if ABLATE != "nodma":
                    nc.sync.dma_start(cur[:, bass.ds(i, TC), :, :], hbf[:])
                else:
                    nc.gpsimd.memset(cur[:, 0, 0, :], 0.0)
            elif ABLATE not in ("nogates", "nodma"):
                if True:
                    nc.vector.tensor_mul(
                        osb[:], hbf[:],
                        mch[:].broadcast_to([128, TC, KC, BS]))
                    nc.sync.dma_start(io["outT"][:, bass.ds(i, TC), :, :],
                                      osb[:])
        if rep_cm is not None:
            rep_cm.__exit__(None, None, None)
    if full_cm is not None:
        full_cm.__exit__(None, None, None)
    ctx.close()


_CACHE = {}


def _get_program():
    if "nc" not in _CACHE:
        _CACHE["nc"] = _build_program()
    return _CACHE["nc"]


def _prep_shared(fc_w, fc_b, ws):
    sh = {}
    sh["fcwT"] = np.ascontiguousarray(
        fc_w.T.reshape(2, 128, HID)).astype(BF)
    sh["fcb"] = np.ascontiguousarray(fc_b.reshape(KC0, 128, 1)).astype(F32)
    for l in range(3):
        w_ih, w_hh, _, _ = ws[l]
        kcl = KC0 if l == 0 else KC
        sh[f"wihT{l}"] = np.ascontiguousarray(
            w_ih.T.reshape(kcl, 128, G3)).astype(BF)
        sh[f"whhT{l}"] = np.ascontiguousarray(
            w_hh.T.reshape(KC, 128, G3)).astype(BF)
    bt = np.zeros((3, 16, 128), F32)
    for l in range(3):
        _, _, b_ih, b_hh = ws[l]
        bi = b_ih.reshape(MCH, 128)
        bh = b_hh.reshape(MCH, 128)
        bt[l, 0:8] = bi[0:8] + bh[0:8]
        bt[l, 8:12] = bi[8:12]
        bt[l, 12:16] = bh[8:12]
    sh["biasT"] = np.broadcast_to(
        bt.reshape(1, -1), (4, 3 * 16 * 128)).astype(BF).copy()
    oh = np.zeros((4, TC, BS), F32)
    for k in range(BS):
        oh[k, :, k] = 1.0
    sh["onehotT"] = oh.reshape(4, TC * BS).astype(BF)
    sh["iota"] = np.broadcast_to(
        np.arange(T, dtype=F32)[None, :], (128, T)).copy()
    return sh


def kernel(z, seq_lens, chord_embedding, fc_w, fc_b,
           w_ih0, w_hh0, b_ih0, b_hh0,
           w_ih1, w_hh1, b_ih1, b_hh1,
           w_ih2, w_hh2, b_ih2, b_hh2):
    z = np.asarray(z, F32)
    chord = np.asarray(chord_embedding, F32)
    seq = np.asarray(seq_lens)
    ws = [(np.asarray(w_ih0, F32), np.asarray(w_hh0, F32),
           np.asarray(b_ih0, F32), np.asarray(b_hh0, F32)),
          (np.asarray(w_ih1, F32), np.asarray(w_hh1, F32),
           np.asarray(b_ih1, F32), np.asarray(b_hh1, F32)),
          (np.asarray(w_ih2, F32), np.asarray(w_hh2, F32),
           np.asarray(b_ih2, F32), np.asarray(b_hh2, F32))]

    in_maps = _make_in_maps(z, seq, chord, np.asarray(fc_w, F32),
                            np.asarray(fc_b, F32), ws)
    res = _execute(in_maps)
    return _assemble(res.results)


def _make_in_maps(z, seq, chord, fc_w, fc_b, ws):
    sh = _prep_shared(fc_w, fc_b, ws)
    in_maps = []
    for c in range(NCORES):
        bs = slice(c * BS, (c + 1) * BS)
        m = dict(sh)
        m["chordT"] = np.ascontiguousarray(
            (chord[bs].transpose(2, 1, 0) / 100.0)
            .reshape(KC0, 128, T, BS)).astype(BF)
        m["zT"] = np.ascontiguousarray(
            z[bs].T.reshape(2, 128, BS)).astype(BF)
        m["seqrep"] = np.broadcast_to(
            seq[bs].astype(F32)[None, :], (128, BS)).copy()
        in_maps.append(m)
    return in_maps


def _execute(in_maps, **kw):
    nc = _get_program()
    return bass_utils.run_bass_kernel_spmd(nc, in_maps, list(range(NCORES)), **kw)


def _assemble(results):
    out = np.empty((B, T, H), F32)
    for c in range(NCORES):
        outT = np.asarray(results[c]["outT"])       # [128,T,KC,BS]
        out[c * BS:(c + 1) * BS] = (
            outT.transpose(3, 1, 2, 0).reshape(BS, T, H))
    return out
